# revision 30
# baseline (speedup 1.0000x reference)
"""Trainium2 Bass kernel for nn_AttentionModel (sparse banded attention).

Math (per batch element, data-parallel over 8 cores):
  qs    = q @ W_score.T
  score = qs @ k.T                      # only the 129-wide causal band matters
  w     = banded_softmax(score)         # full-row max cancels mathematically
  c     = w @ k
  enh   = tanh(concat([c, q]) @ W_enh.T + b_enh)
  out   = sigmoid(enh @ W_mask.T + b_mask)

v2.2 design:
  - bf16 pipeline (PSUM accumulation fp32): halves HBM traffic and weight
    load bandwidth vs fp32r at the same 1 cycle/column matmul rate.
  - T=2000 padded: keys 128 front zeros + 48 tail -> 2176; queries 48 tail
    -> 2048.  Query tile j attends padded key blocks j (prev) and j+1 (diag).
  - kN is host-rearranged to [p, block, h] (contiguous DMA) and shipped in
    fp8e4m3 (it only feeds the fp8 PV matmul).
  - Band mask is MULTIPLICATIVE post-exp: one DVE tensor_tensor_reduce per
    tile computes e*band01 AND the softmax denominator in a single op (no
    PE mask matmuls, no -inf additions).
  - Softmax normalization is fused into the weight transpose: a plain
    matmul  e_half.T @ diag(1/den)  transposes and scales in one PE pass
    (diag built on DVE from the identity by a per-partition scalar mult).
  - PV runs in fp8 DoubleRow: one matmul per (h-chunk, tile) contracts both
    key blocks at 0.5 cycles/column.  w in [0,1] and k ~N(0,1) quantize to
    e4m3 with ~2e-3 end-to-end cost (gate is 2e-2).
  - P3 bias enters PSUM via a K=1 ones-row matmul; sigmoid is computed as
    tanh: W_mask/b_mask are pre-scaled by 0.5 on the host, the device ships
    tanh(z/2) bf16, the host applies 0.5*x+0.5.  P3 activations+outputs are
    processed per tile PAIR (PSUM pair tile spanning 2 banks).
  - Software-pipelined emission: scores(p) | transposes(p-1) | softmax
    chain(p) | PV/P2/P3(p-1) keeps the PE dense while ACT/DVE work.
  - All input DMAs issue up-front across SP/Pool/ACT queues; outputs on SP.
"""

import sys
import types

import numpy as np
from contextlib import ExitStack

import ml_dtypes

import concourse.bass as bass
import concourse.bacc as bacc
import concourse.tile as tile
from concourse import mybir
from concourse.bass_utils import run_bass_kernel_spmd


def _ensure_axon_hooks():
    # bass_utils imports antenv.axon_hooks when tracing is requested; some
    # images lack that module.  Register a shim built from the boot helper
    # so a BASS_TRACE=1 environment doesn't crash the kernel.
    try:
        from antenv import axon_hooks  # noqa: F401
        return
    except ImportError:
        pass
    try:
        from trn_agent_boot.trn_boot import _ntff_profile_via_ctypes
        hook = _ntff_profile_via_ctypes("/opt/axon/libaxon_pjrt.so")
    except Exception:
        hook = None
    m = types.ModuleType("antenv.axon_hooks")
    m.get_axon_ntff_profile_hook = lambda: hook
    m.set_axon_ntff_profile_hook = lambda h: None
    sys.modules["antenv.axon_hooks"] = m


_ensure_axon_hooks()

F32 = mybir.dt.float32
BF = mybir.dt.bfloat16
F8 = mybir.dt.float8e4
AF = mybir.ActivationFunctionType
ALU = mybir.AluOpType
PM = mybir.MatmulPerfMode
BF_NP = ml_dtypes.bfloat16
F8_NP = ml_dtypes.float8_e4m3

B, T, H, F_OUT = 8, 2000, 256, 257
TPK = 2176   # padded key length   (128 front + 2000 + 48 tail)
TPQ = 2048   # padded query length (2000 + 48 tail)
NT = 16      # query tiles of 128
NP = 8       # tile pairs
OPAD = 258   # F_OUT padded to even
N_CORES = 8

_CACHE = {}

import os
USE_FP8_PV = os.environ.get("K_FP8_PV", "1") == "1"
PAIR_TANH = os.environ.get("K_PAIR_TANH", "1") == "1"


def build_nc():
    nc = bacc.Bacc("TRN2", target_bir_lowering=False, debug=False,
                   num_devices=N_CORES)

    kT = nc.declare_dram_parameter("kT", [H, TPK], BF, isOutput=False)
    kN = nc.declare_dram_parameter("kN", [128, 17 * H],
                                   F8 if USE_FP8_PV else BF, isOutput=False)
    qT = nc.declare_dram_parameter("qT", [H, TPQ], BF, isOutput=False)
    wsp = nc.declare_dram_parameter("wsp", [128, 512], BF, isOutput=False)
    wep = nc.declare_dram_parameter("wep", [128, 1024], BF, isOutput=False)
    wmp = nc.declare_dram_parameter("wmp", [128, 2 * OPAD], BF, isOutput=False)
    bmp = nc.declare_dram_parameter("bmp", [1, OPAD], BF, isOutput=False)
    bep = nc.declare_dram_parameter("bep", [128, 2], F32, isOutput=False)
    idn = nc.declare_dram_parameter("idn", [128, 128], BF, isOutput=False)
    mskS = nc.declare_dram_parameter("mskS", [128, 512], BF, isOutput=False)
    msk0 = nc.declare_dram_parameter("msk0", [128, 512], BF, isOutput=False)
    out = nc.declare_dram_parameter("out", [TPQ, OPAD], BF, isOutput=True)

    with tile.TileContext(nc) as tc, ExitStack() as ctx:
        const = ctx.enter_context(tc.tile_pool(name="const", bufs=1))
        io = ctx.enter_context(tc.tile_pool(name="io", bufs=1))
        ep = ctx.enter_context(tc.tile_pool(name="ep", bufs=2))
        wp = ctx.enter_context(tc.tile_pool(name="wp", bufs=2))
        sp_ = ctx.enter_context(tc.tile_pool(name="sp", bufs=2))
        gp = ctx.enter_context(tc.tile_pool(name="gp", bufs=2))
        op_ = ctx.enter_context(tc.tile_pool(name="op", bufs=2))
        pmm = ctx.enter_context(tc.tile_pool(name="pmm", bufs=2, space="PSUM"))
        psc = ctx.enter_context(tc.tile_pool(name="psc", bufs=2, space="PSUM"))
        ppv = ctx.enter_context(tc.tile_pool(name="ppv", bufs=2, space="PSUM"))
        pm3 = ctx.enter_context(tc.tile_pool(
            name="pm3", bufs=1 if PAIR_TANH else 2, space="PSUM"))

        # ---------------- persistent SBUF tiles ----------------
        wst_t = const.tile([128, 512], BF, tag="wst", name="wst_t")
        qT_t = [io.tile([128, TPQ], BF, tag=f"qT{c}", name=f"qT{c}")
                for c in range(2)]
        kT_t = [io.tile([128, TPK], BF, tag=f"kT{c}", name=f"kT{c}")
                for c in range(2)]
        kN_t = io.tile([128, 17 * H], F8 if USE_FP8_PV else BF,
                       tag="kN", name="kN_t")
        idn_t = const.tile([128, 128], BF, tag="idn", name="idn_t")
        mskS_t = const.tile([128, 512], BF, tag="mskS", name="mskS_t")
        msk0_t = const.tile([128, 512], BF, tag="msk0", name="msk0_t")
        wep_t = const.tile([128, 1024], BF, tag="wep", name="wep_t")
        wmp_t = const.tile([128, 2 * OPAD], BF, tag="wmp", name="wmp_t")
        bmp_t = const.tile([1, OPAD], BF, tag="bmp", name="bmp_t")
        bep_t = const.tile([128, 2], F32, tag="bep", name="bep_t")
        ones_t = const.tile([1, 128], BF, tag="ones", name="ones_t")
        qsT_t = [io.tile([128, TPQ], BF, tag=f"qsT{c}", name=f"qsT{c}")
                 for c in range(2)]
        cT_t = io.tile([128, 2 * TPQ], BF, tag="cT", name="cT_t")
        enhT_t = [io.tile([128, TPQ], BF, tag=f"enhT{c}", name=f"enhT{c}")
                  for c in range(2)]

        # ---------------- all loads, issued up-front ----------------
        # The small first-wave pieces are split between the SP and ACT
        # queues so they issue in parallel right after the preamble; all
        # BULK transfers stay on SP/HWDGE (SWDGE bulk bandwidth is poor).
        nc.sync.dma_start(wst_t[:, 0:256], wsp[:, 0:256])
        nc.sync.dma_start(qT_t[0][:, 0:256], qT[0:128, 0:256])
        nc.scalar.dma_start(qT_t[1][:, 0:256], qT[128:256, 0:256])
        nc.sync.dma_start(kT_t[0][:, 0:384], kT[0:128, 0:384])
        nc.scalar.dma_start(kT_t[1][:, 0:384], kT[128:256, 0:384])
        nc.sync.dma_start(wst_t[:, 256:512], wsp[:, 256:512])
        for c in range(2):
            nc.sync.dma_start(qT_t[c][:, 256:2048],
                              qT[c * 128:(c + 1) * 128, 256:2048])
        for c in range(2):
            nc.sync.dma_start(kT_t[c][:, 384:2176],
                              kT[c * 128:(c + 1) * 128, 384:2176])

        # Pool queue (SWDGE, near-free issue): small consts only, so the
        # big kN transfers don't race the critical SP loads for DMA engines.
        nc.gpsimd.dma_start(idn_t[:], idn[:])
        nc.gpsimd.dma_start(mskS_t[:], mskS[:])
        nc.gpsimd.dma_start(msk0_t[:], msk0[:])
        nc.gpsimd.memset(ones_t[:], 1.0)
        # kN on SP after the critical stream; first needed at back(0) ~16us.
        nc.sync.dma_start(kN_t[:, 0:2176], kN[:, 0:2176])
        nc.sync.dma_start(kN_t[:, 2176:4352], kN[:, 2176:4352])

        # ACT queue: P2/P3 weights, needed a few microseconds in.
        nc.scalar.dma_start(wep_t[:], wep[:])
        nc.scalar.dma_start(wmp_t[:], wmp[:])
        nc.scalar.dma_start(bmp_t[:], bmp[:])
        nc.scalar.dma_start(bep_t[:], bep[:])

        # PE warmup: matmuls on a memset scratch tile depend on no DMA, so
        # the tensor engine's DVFS ramp starts right after the preamble and
        # the PE stays busy until the first real operands land.
        wz = const.tile([128, 512], BF, tag="wz", name="wz")
        nc.vector.memset(wz[:], 0.0)
        for r in range(3):
            warm = pmm.tile([128, 512], F32, tag="mm", name="warm")
            nc.tensor.matmul(warm[:], wz[:, 0:128], wz[:],
                             start=True, stop=True)

        # ---------------- compute, software-pipelined over tile pairs ----
        st = {}

        def p0_cols(c0, c1):
            # P0: qsT cols [c0:c1] for both g-chunks.  Emitted one pair
            # AHEAD of its consumers so the PSUM->SBUF cast never stalls
            # the scores matmuls.
            w = c1 - c0
            for gc in range(2):
                ps0 = pmm.tile([128, 512], F32, tag="mm", name="ps0")
                for hc in range(2):
                    nc.tensor.matmul(
                        ps0[:, 0:w],
                        wst_t[:, hc * 256 + gc * 128: hc * 256 + gc * 128 + 128],
                        qT_t[hc][:, c0:c1],
                        start=(hc == 0), stop=(hc == 1))
                nc.vector.tensor_copy(qsT_t[gc][:, c0:c1], ps0[:, 0:w])

        def p0_chunk(nb):
            p0_cols(nb * 512, (nb + 1) * 512)

        def front(p):
            # P1 scores for tiles 2p, 2p+1 in one PSUM bank
            sc = psc.tile([128, 512], F32, tag="sc", name="sc")
            nc.tensor.matmul(sc[:], idn_t[:],
                             (msk0_t if p == 0 else mskS_t)[:],
                             start=True, stop=False)
            for l in range(2):
                j = 2 * p + l
                for gc in range(2):
                    nc.tensor.matmul(
                        sc[:, l * 256:(l + 1) * 256],
                        qsT_t[gc][:, j * 128:(j + 1) * 128],
                        kT_t[gc][:, j * 128: j * 128 + 256],
                        start=False, stop=(gc == 1))
            st[p] = {"sc": sc}

        def chain(p):
            # exp; band mask (multiplicative) + row sums fused; 1/den; diag
            s = st[p]
            e_t = ep.tile([128, 512], BF, tag="e", name="e_t")
            nc.scalar.activation(e_t[:], s["sc"][:], AF.Exp)
            den = sp_.tile([128, 2], F32, tag="den", name="den")
            nc.vector.reduce_sum(
                den[:], e_t[:].rearrange("p (l s) -> p l s", s=256),
                axis=mybir.AxisListType.X)
            rec = sp_.tile([128, 2], F32, tag="rec", name="rec")
            nc.vector.reciprocal(rec[:], den[:])
            dg = gp.tile([128, 256], BF, tag="dg", name="dg")
            for l in range(2):
                nc.vector.tensor_scalar_mul(
                    dg[:, l * 128:(l + 1) * 128], idn_t[:], rec[:, l:l + 1])
            s["em"] = e_t
            s["dg"] = dg

        def transp(p):
            # transpose + normalize fused:  pw = em_half.T @ diag(rec)
            s = st[p]
            pw = ppv.tile([128, 512], F32, tag="pv", name="pw")
            for l in range(2):
                for hf in range(2):
                    r = (l * 2 + hf) * 128
                    nc.tensor.matmul(
                        pw[:, r:r + 128],
                        s["em"][:, l * 256 + hf * 128: l * 256 + hf * 128 + 128],
                        s["dg"][:, l * 128:(l + 1) * 128],
                        start=True, stop=True)
            wt = wp.tile([128, 512], F8 if USE_FP8_PV else BF,
                     tag="wt", name="wt")
            nc.vector.tensor_copy(wt[:], pw[:])
            s["wt"] = wt

        kN_v = kN_t[:].rearrange("p (b h) -> p b h", h=256)

        def back(p):
            s = st.pop(p)
            wt = s["wt"]
            # PV in fp8 DoubleRow: per (h-chunk, tile) one matmul contracts
            # both key blocks.  pc layout: [t0h0 | t1h0 | t0h1 | t1h1]
            pc = ppv.tile([128, 512], F32, tag="pv", name="pc")
            for hc in range(2):
                for l in range(2):
                    j = 2 * p + l
                    if USE_FP8_PV:
                        nc.tensor.matmul(
                            pc[:, hc * 256 + l * 128: hc * 256 + l * 128 + 128],
                            kN_v[:, j:j + 2, hc * 128:hc * 128 + 128],
                            wt[:, l * 256:(l + 1) * 256].rearrange(
                                "p (b t) -> p b t", t=128),
                            start=True, stop=True, perf_mode=PM.DoubleRow)
                    else:
                        for hf in range(2):
                            nc.tensor.matmul(
                                pc[:, hc * 256 + l * 128: hc * 256 + l * 128 + 128],
                                kN_v[:, j + hf, hc * 128:hc * 128 + 128],
                                wt[:, (l * 2 + hf) * 128:(l * 2 + hf) * 128 + 128],
                                start=(hf == 0), stop=(hf == 1))
            nc.scalar.copy(
                cT_t[:].rearrange("p (h t) -> p h t", t=TPQ)
                [:, :, p * 256:(p + 1) * 256],
                pc[:].rearrange("p (h t) -> p h t", t=256))

            # P2: enhT = tanh(W_enh.T @ [c; q] + b_enh) for the pair
            pe2 = pmm.tile([128, 512], F32, tag="mm", name="pe2")
            for f in range(2):
                for dc in range(4):
                    rhs = (cT_t[:, dc * TPQ + p * 256: dc * TPQ + (p + 1) * 256]
                           if dc < 2 else
                           qT_t[dc - 2][:, p * 256:(p + 1) * 256])
                    nc.tensor.matmul(
                        pe2[:, f * 256:(f + 1) * 256],
                        wep_t[:, dc * 256 + f * 128: dc * 256 + f * 128 + 128],
                        rhs,
                        start=(dc == 0), stop=(dc == 3))
            for f in range(2):
                nc.scalar.activation(enhT_t[f][:, p * 256:(p + 1) * 256],
                                     pe2[:, f * 256:(f + 1) * 256],
                                     AF.Tanh, bias=bep_t[:, f:f + 1])

            # P3: out = tanh(enh @ (W_mask.T/2) + b_mask/2)
            if PAIR_TANH:
                pm = pm3.tile([128, 1024], F32, tag="p3", name="pm")
                ot = op_.tile([128, 2 * OPAD], BF, tag="ot", name="ot")
                if p == NP - 1:
                    # last pair: per-tile tanh+DMA so tile 14's output ships
                    # while tile 15's matmuls still run (shorter tail)
                    for l in range(2):
                        j = 2 * p + l
                        r = l * 512
                        nc.tensor.matmul(pm[:, r:r + OPAD], ones_t[:],
                                         bmp_t[:], start=True, stop=False)
                        for f in range(2):
                            nc.tensor.matmul(
                                pm[:, r:r + OPAD],
                                enhT_t[f][:, j * 128:(j + 1) * 128],
                                wmp_t[:, f * OPAD:(f + 1) * OPAD],
                                start=False, stop=(f == 1))
                        nc.scalar.activation(ot[:, l * OPAD:(l + 1) * OPAD],
                                             pm[:, r:r + OPAD], AF.Tanh)
                        nc.sync.dma_start(out[j * 128:(j + 1) * 128, :],
                                          ot[:, l * OPAD:(l + 1) * OPAD])
                else:
                    # pair PSUM tile spanning 2 banks; ONE strided tanh
                    for l in range(2):
                        j = 2 * p + l
                        r = l * 512
                        nc.tensor.matmul(pm[:, r:r + OPAD], ones_t[:],
                                         bmp_t[:], start=True, stop=False)
                        for f in range(2):
                            nc.tensor.matmul(
                                pm[:, r:r + OPAD],
                                enhT_t[f][:, j * 128:(j + 1) * 128],
                                wmp_t[:, f * OPAD:(f + 1) * OPAD],
                                start=False, stop=(f == 1))
                    nc.scalar.activation(
                        ot[:].rearrange("p (l o) -> p l o", o=OPAD),
                        pm[:].rearrange("p (l x) -> p l x", x=512)[:, :, 0:OPAD],
                        AF.Tanh)
                    nc.sync.dma_start(
                        out[p * 256:(p + 1) * 256, :].rearrange(
                            "(l r) o -> r l o", r=128),
                        ot[:].rearrange("p (l o) -> p l o", o=OPAD))
            else:
                for l in range(2):
                    j = 2 * p + l
                    pm = pm3.tile([128, OPAD], F32, tag="p3", name="pm")
                    nc.tensor.matmul(pm[:], ones_t[:], bmp_t[:],
                                     start=True, stop=False)
                    for f in range(2):
                        nc.tensor.matmul(
                            pm[:],
                            enhT_t[f][:, j * 128:(j + 1) * 128],
                            wmp_t[:, f * OPAD:(f + 1) * OPAD],
                            start=False, stop=(f == 1))
                    ot = op_.tile([128, OPAD], BF, tag="ot", name="ot")
                    nc.scalar.activation(ot[:], pm[:], AF.Tanh)
                    nc.sync.dma_start(out[j * 128:(j + 1) * 128, :], ot[:])

        p0_cols(0, 256)
        p0_cols(256, 512)
        front(0)
        chain(0)
        for p in range(1, NP):
            front(p)
            transp(p - 1)
            if p % 2 == 1 and p < NP - 1:
                p0_chunk((p + 1) // 2)
            chain(p)
            back(p - 1)
        transp(NP - 1)
        back(NP - 1)

    return nc


def _prep_shared(W_score, W_enh, b_enh, W_mask, b_mask):
    # wsp: W_score.T [h, g] packed [128, (hc, g)]
    WsT = W_score.T.astype(np.float32)                     # [h, g]
    wsp = np.ascontiguousarray(
        WsT.reshape(2, 128, 256).transpose(1, 0, 2).reshape(128, 512)
    ).astype(BF_NP)
    # wep: W_enh.T [d, f] packed [128, (dc, f)]
    WeT = W_enh.T.astype(np.float32)                       # [512, 256]
    wep = np.ascontiguousarray(
        WeT.reshape(4, 128, 256).transpose(1, 0, 2).reshape(128, 1024)
    ).astype(BF_NP)
    # wmp: 0.5 * W_mask.T [f, o] padded to OPAD, packed [128, (fc, OPAD)]
    WmT = np.zeros((H, OPAD), np.float32)
    WmT[:, :F_OUT] = 0.5 * W_mask.T.astype(np.float32)
    wmp = np.ascontiguousarray(
        WmT.reshape(2, 128, OPAD).transpose(1, 0, 2).reshape(128, 2 * OPAD)
    ).astype(BF_NP)
    bmp = np.zeros((1, OPAD), np.float32)
    bmp[0, :F_OUT] = 0.5 * b_mask.astype(np.float32)
    bmp = bmp.astype(BF_NP)
    bep = np.ascontiguousarray(
        b_enh.astype(np.float32).reshape(2, 128).T)        # [128, 2]
    # additive band masks + identity
    NEG = -32768.0
    t_i = np.arange(128, dtype=np.int32)[:, None]
    s_i = np.arange(128, dtype=np.int32)[None, :]
    mask_prev = np.where(s_i >= t_i, 0.0, NEG).astype(np.float32)
    mask_diag = np.where(s_i <= t_i, 0.0, NEG).astype(np.float32)
    tile_std = np.concatenate([mask_prev, mask_diag], 1)
    tile_t0 = np.concatenate(
        [np.full((128, 128), NEG, np.float32), mask_diag], 1)
    mskS = np.ascontiguousarray(
        np.concatenate([tile_std, tile_std], 1)).astype(BF_NP)
    msk0 = np.ascontiguousarray(
        np.concatenate([tile_t0, tile_std], 1)).astype(BF_NP)
    idn = np.eye(128, dtype=np.float32).astype(BF_NP)
    return wsp, wep, wmp, bmp, bep, mskS, msk0, idn


def make_in_maps(k, q, W_score, W_enh, b_enh, W_mask, b_mask):
    k = np.asarray(k, np.float32)
    q = np.asarray(q, np.float32)
    wsp, wep, wmp, bmp, bep, mskS, msk0, idn = _prep_shared(
        np.asarray(W_score, np.float32), np.asarray(W_enh, np.float32),
        np.asarray(b_enh, np.float32), np.asarray(W_mask, np.float32),
        np.asarray(b_mask, np.float32))
    in_maps = []
    for b in range(N_CORES):
        kb = np.zeros((TPK, H), np.float32)
        kb[128:128 + T] = k[b]
        qb = np.zeros((TPQ, H), np.float32)
        qb[:T] = q[b]
        in_maps.append({
            "kT": np.ascontiguousarray(kb.astype(BF_NP).T),
            "kN": np.ascontiguousarray(
                kb.reshape(17, 128, H).transpose(1, 0, 2).reshape(128, 17 * H)
            ).astype(F8_NP if USE_FP8_PV else BF_NP),
            "qT": np.ascontiguousarray(qb.astype(BF_NP).T),
            "wsp": wsp, "wep": wep, "wmp": wmp, "bmp": bmp, "bep": bep,
            "mskS": mskS, "msk0": msk0, "idn": idn,
        })
    return in_maps


def get_nc():
    if "nc" not in _CACHE:
        nc = build_nc()
        nc.finalize()
        _CACHE["nc"] = nc
    return _CACHE["nc"]


def postprocess(results):
    outs = []
    for r in results:
        o = np.asarray(r["out"]).astype(np.float32)
        outs.append(0.5 * o[:T, :F_OUT] + 0.5)
    return np.stack(outs, 0)


def kernel(k, q, W_score, W_enh, b_enh, W_mask, b_mask):
    in_maps = make_in_maps(k, q, W_score, W_enh, b_enh, W_mask, b_mask)
    res = run_bass_kernel_spmd(get_nc(), in_maps, list(range(N_CORES)))
    return postprocess(res.results)


# revision 32
# speedup vs baseline: 1.2031x; 1.2031x over previous
"""Trainium2 Bass kernel for nn_AttentionModel (sparse banded attention).

Math (per batch element, data-parallel over 8 cores):
  qs    = q @ W_score.T
  score = qs @ k.T                      # only the 129-wide causal band matters
  w     = banded_softmax(score)         # full-row max cancels mathematically
  c     = w @ k
  enh   = tanh(concat([c, q]) @ W_enh.T + b_enh)
  out   = sigmoid(enh @ W_mask.T + b_mask)

v2.2 design:
  - bf16 pipeline (PSUM accumulation fp32): halves HBM traffic and weight
    load bandwidth vs fp32r at the same 1 cycle/column matmul rate.
  - T=2000 padded: keys 128 front zeros + 48 tail -> 2176; queries 48 tail
    -> 2048.  Query tile j attends padded key blocks j (prev) and j+1 (diag).
  - kN is host-rearranged to [p, block, h] (contiguous DMA) and shipped in
    fp8e4m3 (it only feeds the fp8 PV matmul).
  - Band mask is MULTIPLICATIVE post-exp: one DVE tensor_tensor_reduce per
    tile computes e*band01 AND the softmax denominator in a single op (no
    PE mask matmuls, no -inf additions).
  - Softmax normalization is fused into the weight transpose: a plain
    matmul  e_half.T @ diag(1/den)  transposes and scales in one PE pass
    (diag built on DVE from the identity by a per-partition scalar mult).
  - PV runs in fp8 DoubleRow: one matmul per (h-chunk, tile) contracts both
    key blocks at 0.5 cycles/column.  w in [0,1] and k ~N(0,1) quantize to
    e4m3 with ~2e-3 end-to-end cost (gate is 2e-2).
  - P3 bias enters PSUM via a K=1 ones-row matmul; sigmoid is computed as
    tanh: W_mask/b_mask are pre-scaled by 0.5 on the host, the device ships
    tanh(z/2) bf16, the host applies 0.5*x+0.5.  P3 activations+outputs are
    processed per tile PAIR (PSUM pair tile spanning 2 banks).
  - Software-pipelined emission: scores(p) | transposes(p-1) | softmax
    chain(p) | PV/P2/P3(p-1) keeps the PE dense while ACT/DVE work.
  - All input DMAs issue up-front across SP/Pool/ACT queues; outputs on SP.
"""

import sys
import types

import numpy as np
from contextlib import ExitStack

import ml_dtypes

import concourse.bass as bass
import concourse.bacc as bacc
import concourse.tile as tile
from concourse import mybir
from concourse.bass_utils import run_bass_kernel_spmd


def _ensure_axon_hooks():
    # bass_utils imports antenv.axon_hooks when tracing is requested; some
    # images lack that module.  Register a shim built from the boot helper
    # so a BASS_TRACE=1 environment doesn't crash the kernel.
    try:
        from antenv import axon_hooks  # noqa: F401
        return
    except ImportError:
        pass
    try:
        from trn_agent_boot.trn_boot import _ntff_profile_via_ctypes
        hook = _ntff_profile_via_ctypes("/opt/axon/libaxon_pjrt.so")
    except Exception:
        hook = None
    m = types.ModuleType("antenv.axon_hooks")
    m.get_axon_ntff_profile_hook = lambda: hook
    m.set_axon_ntff_profile_hook = lambda h: None
    sys.modules["antenv.axon_hooks"] = m


_ensure_axon_hooks()

F32 = mybir.dt.float32
BF = mybir.dt.bfloat16
F8 = mybir.dt.float8e4
AF = mybir.ActivationFunctionType
ALU = mybir.AluOpType
PM = mybir.MatmulPerfMode
BF_NP = ml_dtypes.bfloat16
F8_NP = ml_dtypes.float8_e4m3

B, T, H, F_OUT = 8, 2000, 256, 257
TPK = 2176   # padded key length   (128 front + 2000 + 48 tail)
TPQ = 2048   # padded query length (2000 + 48 tail)
NT = 16      # query tiles of 128
NP = 8       # tile pairs
OPAD = 258   # F_OUT padded to even
N_CORES = 8

_CACHE = {}

import os
USE_FP8_PV = os.environ.get("K_FP8_PV", "1") == "1"
PAIR_TANH = os.environ.get("K_PAIR_TANH", "1") == "1"


def build_nc():
    nc = bacc.Bacc("TRN2", target_bir_lowering=False, debug=False,
                   num_devices=N_CORES)

    kT = nc.declare_dram_parameter("kT", [H, TPK], BF, isOutput=False)
    kN = nc.declare_dram_parameter("kN", [128, 17 * H],
                                   F8 if USE_FP8_PV else BF, isOutput=False)
    qT = nc.declare_dram_parameter("qT", [H, TPQ], BF, isOutput=False)
    wsp = nc.declare_dram_parameter("wsp", [128, 512], BF, isOutput=False)
    wep = nc.declare_dram_parameter("wep", [128, 1024], BF, isOutput=False)
    wmp = nc.declare_dram_parameter("wmp", [128, 2 * OPAD], BF, isOutput=False)
    bmp = nc.declare_dram_parameter("bmp", [1, OPAD], BF, isOutput=False)
    bep = nc.declare_dram_parameter("bep", [128, 2], F32, isOutput=False)
    idn = nc.declare_dram_parameter("idn", [128, 128], BF, isOutput=False)
    mskS = nc.declare_dram_parameter("mskS", [128, 512], BF, isOutput=False)
    msk0 = nc.declare_dram_parameter("msk0", [128, 512], BF, isOutput=False)
    out = nc.declare_dram_parameter("out", [TPQ, OPAD], BF, isOutput=True)

    with tile.TileContext(nc) as tc, ExitStack() as ctx:
        const = ctx.enter_context(tc.tile_pool(name="const", bufs=1))
        io = ctx.enter_context(tc.tile_pool(name="io", bufs=1))
        ep = ctx.enter_context(tc.tile_pool(name="ep", bufs=2))
        wp = ctx.enter_context(tc.tile_pool(name="wp", bufs=2))
        sp_ = ctx.enter_context(tc.tile_pool(name="sp", bufs=2))
        gp = ctx.enter_context(tc.tile_pool(name="gp", bufs=2))
        op_ = ctx.enter_context(tc.tile_pool(name="op", bufs=2))
        pmm = ctx.enter_context(tc.tile_pool(name="pmm", bufs=2, space="PSUM"))
        psc = ctx.enter_context(tc.tile_pool(name="psc", bufs=2, space="PSUM"))
        ppv = ctx.enter_context(tc.tile_pool(name="ppv", bufs=2, space="PSUM"))
        pm3 = ctx.enter_context(tc.tile_pool(
            name="pm3", bufs=1 if PAIR_TANH else 2, space="PSUM"))

        # ---------------- persistent SBUF tiles ----------------
        wst_t = const.tile([128, 512], BF, tag="wst", name="wst_t")
        qT_t = [io.tile([128, TPQ], BF, tag=f"qT{c}", name=f"qT{c}")
                for c in range(2)]
        kT_t = [io.tile([128, TPK], BF, tag=f"kT{c}", name=f"kT{c}")
                for c in range(2)]
        kN_t = io.tile([128, 17 * H], F8 if USE_FP8_PV else BF,
                       tag="kN", name="kN_t")
        idn_t = const.tile([128, 128], BF, tag="idn", name="idn_t")
        mskS_t = const.tile([128, 512], BF, tag="mskS", name="mskS_t")
        msk0_t = const.tile([128, 512], BF, tag="msk0", name="msk0_t")
        wep_t = const.tile([128, 1024], BF, tag="wep", name="wep_t")
        wmp_t = const.tile([128, 2 * OPAD], BF, tag="wmp", name="wmp_t")
        bmp_t = const.tile([1, OPAD], BF, tag="bmp", name="bmp_t")
        bep_t = const.tile([128, 2], F32, tag="bep", name="bep_t")
        ones_t = const.tile([1, 128], BF, tag="ones", name="ones_t")
        qsT_t = [io.tile([128, TPQ], BF, tag=f"qsT{c}", name=f"qsT{c}")
                 for c in range(2)]
        cT_t = io.tile([128, 2 * TPQ], BF, tag="cT", name="cT_t")
        enhT_t = [io.tile([128, TPQ], BF, tag=f"enhT{c}", name=f"enhT{c}")
                  for c in range(2)]

        # ---------------- all loads, issued up-front ----------------
        # The small first-wave pieces are split between the SP and ACT
        # queues so they issue in parallel right after the preamble; all
        # BULK transfers stay on SP/HWDGE (SWDGE bulk bandwidth is poor).
        nc.sync.dma_start(wst_t[:, 0:256], wsp[:, 0:256])
        nc.sync.dma_start(qT_t[0][:, 0:256], qT[0:128, 0:256])
        nc.scalar.dma_start(qT_t[1][:, 0:256], qT[128:256, 0:256])
        nc.sync.dma_start(kT_t[0][:, 0:384], kT[0:128, 0:384])
        nc.scalar.dma_start(kT_t[1][:, 0:384], kT[128:256, 0:384])
        nc.sync.dma_start(wst_t[:, 256:512], wsp[:, 256:512])
        for c in range(2):
            nc.sync.dma_start(qT_t[c][:, 256:2048],
                              qT[c * 128:(c + 1) * 128, 256:2048])
        for c in range(2):
            nc.sync.dma_start(kT_t[c][:, 384:2176],
                              kT[c * 128:(c + 1) * 128, 384:2176])

        # Pool queue (SWDGE, near-free issue): small consts only, so the
        # big kN transfers don't race the critical SP loads for DMA engines.
        nc.gpsimd.dma_start(idn_t[:], idn[:])
        nc.gpsimd.dma_start(mskS_t[:], mskS[:])
        nc.gpsimd.dma_start(msk0_t[:], msk0[:])
        nc.gpsimd.memset(ones_t[:], 1.0)
        # kN on SP after the critical stream; first needed at back(0) ~16us.
        nc.sync.dma_start(kN_t[:, 0:2176], kN[:, 0:2176])
        nc.sync.dma_start(kN_t[:, 2176:4352], kN[:, 2176:4352])

        # ACT queue: P2/P3 weights, needed a few microseconds in.
        nc.scalar.dma_start(wep_t[:], wep[:])
        nc.scalar.dma_start(wmp_t[:], wmp[:])
        nc.scalar.dma_start(bmp_t[:], bmp[:])
        nc.scalar.dma_start(bep_t[:], bep[:])

        # PE warmup: matmuls on a memset scratch tile depend on no DMA, so
        # the tensor engine's DVFS ramp starts right after the preamble and
        # the PE stays busy until the first real operands land.
        wz = const.tile([128, 512], BF, tag="wz", name="wz")
        nc.vector.memset(wz[:], 0.0)
        for r in range(3):
            warm = pmm.tile([128, 512], F32, tag="mm", name="warm")
            nc.tensor.matmul(warm[:], wz[:, 0:128], wz[:],
                             start=True, stop=True)

        # ---------------- compute, software-pipelined over tile pairs ----
        st = {}

        def p0_cols(c0, c1):
            # P0: qsT cols [c0:c1] for both g-chunks.  Emitted one pair
            # AHEAD of its consumers so the PSUM->SBUF cast never stalls
            # the scores matmuls.
            w = c1 - c0
            for gc in range(2):
                ps0 = pmm.tile([128, 512], F32, tag="mm", name="ps0")
                for hc in range(2):
                    nc.tensor.matmul(
                        ps0[:, 0:w],
                        wst_t[:, hc * 256 + gc * 128: hc * 256 + gc * 128 + 128],
                        qT_t[hc][:, c0:c1],
                        start=(hc == 0), stop=(hc == 1))
                nc.vector.tensor_copy(qsT_t[gc][:, c0:c1], ps0[:, 0:w])

        def p0_chunk(nb):
            p0_cols(nb * 512, (nb + 1) * 512)

        def front(p):
            # P1 scores for tiles 2p, 2p+1 in one PSUM bank
            sc = psc.tile([128, 512], F32, tag="sc", name="sc")
            nc.tensor.matmul(sc[:], idn_t[:],
                             (msk0_t if p == 0 else mskS_t)[:],
                             start=True, stop=False)
            for l in range(2):
                j = 2 * p + l
                for gc in range(2):
                    nc.tensor.matmul(
                        sc[:, l * 256:(l + 1) * 256],
                        qsT_t[gc][:, j * 128:(j + 1) * 128],
                        kT_t[gc][:, j * 128: j * 128 + 256],
                        start=False, stop=(gc == 1))
            st[p] = {"sc": sc}

        def chain(p):
            # exp; band mask (multiplicative) + row sums fused; 1/den; diag
            s = st[p]
            e_t = ep.tile([128, 512], BF, tag="e", name="e_t")
            nc.scalar.activation(e_t[:], s["sc"][:], AF.Exp)
            den = sp_.tile([128, 2], F32, tag="den", name="den")
            nc.vector.reduce_sum(
                den[:], e_t[:].rearrange("p (l s) -> p l s", s=256),
                axis=mybir.AxisListType.X)
            rec = sp_.tile([128, 2], F32, tag="rec", name="rec")
            nc.vector.reciprocal(rec[:], den[:])
            dg = gp.tile([128, 256], BF, tag="dg", name="dg")
            for l in range(2):
                nc.vector.tensor_scalar_mul(
                    dg[:, l * 128:(l + 1) * 128], idn_t[:], rec[:, l:l + 1])
            s["em"] = e_t
            s["dg"] = dg

        def transp(p):
            # transpose + normalize fused:  pw = em_half.T @ diag(rec)
            s = st[p]
            pw = ppv.tile([128, 512], F32, tag="pv", name="pw")
            for l in range(2):
                for hf in range(2):
                    r = (l * 2 + hf) * 128
                    nc.tensor.matmul(
                        pw[:, r:r + 128],
                        s["em"][:, l * 256 + hf * 128: l * 256 + hf * 128 + 128],
                        s["dg"][:, l * 128:(l + 1) * 128],
                        start=True, stop=True)
            wt = wp.tile([128, 512], F8 if USE_FP8_PV else BF,
                     tag="wt", name="wt")
            nc.vector.tensor_copy(wt[:], pw[:])
            s["wt"] = wt

        kN_v = kN_t[:].rearrange("p (b h) -> p b h", h=256)

        def back(p):
            s = st.pop(p)
            wt = s["wt"]
            # PV in fp8 DoubleRow: per (h-chunk, tile) one matmul contracts
            # both key blocks.  pc layout: [t0h0 | t1h0 | t0h1 | t1h1]
            pc = ppv.tile([128, 512], F32, tag="pv", name="pc")
            for hc in range(2):
                for l in range(2):
                    j = 2 * p + l
                    if USE_FP8_PV:
                        nc.tensor.matmul(
                            pc[:, hc * 256 + l * 128: hc * 256 + l * 128 + 128],
                            kN_v[:, j:j + 2, hc * 128:hc * 128 + 128],
                            wt[:, l * 256:(l + 1) * 256].rearrange(
                                "p (b t) -> p b t", t=128),
                            start=True, stop=True, perf_mode=PM.DoubleRow)
                    else:
                        for hf in range(2):
                            nc.tensor.matmul(
                                pc[:, hc * 256 + l * 128: hc * 256 + l * 128 + 128],
                                kN_v[:, j + hf, hc * 128:hc * 128 + 128],
                                wt[:, (l * 2 + hf) * 128:(l * 2 + hf) * 128 + 128],
                                start=(hf == 0), stop=(hf == 1))
            nc.scalar.copy(
                cT_t[:].rearrange("p (h t) -> p h t", t=TPQ)
                [:, :, p * 256:(p + 1) * 256],
                pc[:].rearrange("p (h t) -> p h t", t=256))

            # P2: enhT = tanh(W_enh.T @ [c; q] + b_enh) for the pair
            pe2 = pmm.tile([128, 512], F32, tag="mm", name="pe2")
            for f in range(2):
                for dc in range(4):
                    rhs = (cT_t[:, dc * TPQ + p * 256: dc * TPQ + (p + 1) * 256]
                           if dc < 2 else
                           qT_t[dc - 2][:, p * 256:(p + 1) * 256])
                    nc.tensor.matmul(
                        pe2[:, f * 256:(f + 1) * 256],
                        wep_t[:, dc * 256 + f * 128: dc * 256 + f * 128 + 128],
                        rhs,
                        start=(dc == 0), stop=(dc == 3))
            for f in range(2):
                nc.scalar.activation(enhT_t[f][:, p * 256:(p + 1) * 256],
                                     pe2[:, f * 256:(f + 1) * 256],
                                     AF.Tanh, bias=bep_t[:, f:f + 1])

            # P3: out = tanh(enh @ (W_mask.T/2) + b_mask/2)
            if PAIR_TANH:
                pm = pm3.tile([128, 1024], F32, tag="p3", name="pm")
                ot = op_.tile([128, 2 * OPAD], BF, tag="ot", name="ot")
                if p == NP - 1:
                    # last pair: per-tile tanh+DMA so tile 14's output ships
                    # while tile 15's matmuls still run (shorter tail)
                    for l in range(2):
                        j = 2 * p + l
                        r = l * 512
                        nc.tensor.matmul(pm[:, r:r + OPAD], ones_t[:],
                                         bmp_t[:], start=True, stop=False)
                        for f in range(2):
                            nc.tensor.matmul(
                                pm[:, r:r + OPAD],
                                enhT_t[f][:, j * 128:(j + 1) * 128],
                                wmp_t[:, f * OPAD:(f + 1) * OPAD],
                                start=False, stop=(f == 1))
                        nc.scalar.activation(ot[:, l * OPAD:(l + 1) * OPAD],
                                             pm[:, r:r + OPAD], AF.Tanh)
                        nc.sync.dma_start(out[j * 128:(j + 1) * 128, :],
                                          ot[:, l * OPAD:(l + 1) * OPAD])
                else:
                    # pair PSUM tile spanning 2 banks; ONE strided tanh
                    for l in range(2):
                        j = 2 * p + l
                        r = l * 512
                        nc.tensor.matmul(pm[:, r:r + OPAD], ones_t[:],
                                         bmp_t[:], start=True, stop=False)
                        for f in range(2):
                            nc.tensor.matmul(
                                pm[:, r:r + OPAD],
                                enhT_t[f][:, j * 128:(j + 1) * 128],
                                wmp_t[:, f * OPAD:(f + 1) * OPAD],
                                start=False, stop=(f == 1))
                    nc.scalar.activation(
                        ot[:].rearrange("p (l o) -> p l o", o=OPAD),
                        pm[:].rearrange("p (l x) -> p l x", x=512)[:, :, 0:OPAD],
                        AF.Tanh)
                    nc.sync.dma_start(
                        out[p * 256:(p + 1) * 256, :].rearrange(
                            "(l r) o -> r l o", r=128),
                        ot[:].rearrange("p (l o) -> p l o", o=OPAD))
            else:
                for l in range(2):
                    j = 2 * p + l
                    pm = pm3.tile([128, OPAD], F32, tag="p3", name="pm")
                    nc.tensor.matmul(pm[:], ones_t[:], bmp_t[:],
                                     start=True, stop=False)
                    for f in range(2):
                        nc.tensor.matmul(
                            pm[:],
                            enhT_t[f][:, j * 128:(j + 1) * 128],
                            wmp_t[:, f * OPAD:(f + 1) * OPAD],
                            start=False, stop=(f == 1))
                    ot = op_.tile([128, OPAD], BF, tag="ot", name="ot")
                    nc.scalar.activation(ot[:], pm[:], AF.Tanh)
                    nc.sync.dma_start(out[j * 128:(j + 1) * 128, :], ot[:])

        p0_cols(0, 256)
        p0_cols(256, 512)
        front(0)
        chain(0)
        for p in range(1, NP):
            front(p)
            transp(p - 1)
            if p % 2 == 1 and p < NP - 1:
                p0_chunk((p + 1) // 2)
            chain(p)
            back(p - 1)
        transp(NP - 1)
        back(NP - 1)

    return nc


def _prep_shared(W_score, W_enh, b_enh, W_mask, b_mask):
    # wsp: W_score.T [h, g] packed [128, (hc, g)]
    WsT = W_score.T.astype(np.float32)                     # [h, g]
    wsp = np.ascontiguousarray(
        WsT.reshape(2, 128, 256).transpose(1, 0, 2).reshape(128, 512)
    ).astype(BF_NP)
    # wep: W_enh.T [d, f] packed [128, (dc, f)]
    WeT = W_enh.T.astype(np.float32)                       # [512, 256]
    wep = np.ascontiguousarray(
        WeT.reshape(4, 128, 256).transpose(1, 0, 2).reshape(128, 1024)
    ).astype(BF_NP)
    # wmp: 0.5 * W_mask.T [f, o] padded to OPAD, packed [128, (fc, OPAD)]
    WmT = np.zeros((H, OPAD), np.float32)
    WmT[:, :F_OUT] = 0.5 * W_mask.T.astype(np.float32)
    wmp = np.ascontiguousarray(
        WmT.reshape(2, 128, OPAD).transpose(1, 0, 2).reshape(128, 2 * OPAD)
    ).astype(BF_NP)
    bmp = np.zeros((1, OPAD), np.float32)
    bmp[0, :F_OUT] = 0.5 * b_mask.astype(np.float32)
    bmp = bmp.astype(BF_NP)
    bep = np.ascontiguousarray(
        b_enh.astype(np.float32).reshape(2, 128).T)        # [128, 2]
    # additive band masks + identity
    NEG = -32768.0
    t_i = np.arange(128, dtype=np.int32)[:, None]
    s_i = np.arange(128, dtype=np.int32)[None, :]
    mask_prev = np.where(s_i >= t_i, 0.0, NEG).astype(np.float32)
    mask_diag = np.where(s_i <= t_i, 0.0, NEG).astype(np.float32)
    tile_std = np.concatenate([mask_prev, mask_diag], 1)
    tile_t0 = np.concatenate(
        [np.full((128, 128), NEG, np.float32), mask_diag], 1)
    mskS = np.ascontiguousarray(
        np.concatenate([tile_std, tile_std], 1)).astype(BF_NP)
    msk0 = np.ascontiguousarray(
        np.concatenate([tile_t0, tile_std], 1)).astype(BF_NP)
    idn = np.eye(128, dtype=np.float32).astype(BF_NP)
    return wsp, wep, wmp, bmp, bep, mskS, msk0, idn


def make_in_maps(k, q, W_score, W_enh, b_enh, W_mask, b_mask):
    k = np.asarray(k, np.float32)
    q = np.asarray(q, np.float32)
    wsp, wep, wmp, bmp, bep, mskS, msk0, idn = _prep_shared(
        np.asarray(W_score, np.float32), np.asarray(W_enh, np.float32),
        np.asarray(b_enh, np.float32), np.asarray(W_mask, np.float32),
        np.asarray(b_mask, np.float32))
    in_maps = []
    for b in range(N_CORES):
        kb = np.zeros((TPK, H), np.float32)
        kb[128:128 + T] = k[b]
        qb = np.zeros((TPQ, H), np.float32)
        qb[:T] = q[b]
        in_maps.append({
            "kT": np.ascontiguousarray(kb.astype(BF_NP).T),
            "kN": np.ascontiguousarray(
                kb.reshape(17, 128, H).transpose(1, 0, 2).reshape(128, 17 * H)
            ).astype(F8_NP if USE_FP8_PV else BF_NP),
            "qT": np.ascontiguousarray(qb.astype(BF_NP).T),
            "wsp": wsp, "wep": wep, "wmp": wmp, "bmp": bmp, "bep": bep,
            "mskS": mskS, "msk0": msk0, "idn": idn,
        })
    return in_maps


def get_nc():
    if "nc" not in _CACHE:
        nc = build_nc()
        nc.finalize()
        _CACHE["nc"] = nc
    return _CACHE["nc"]


def postprocess(results):
    outs = []
    for r in results:
        o = np.asarray(r["out"]).astype(np.float32)
        outs.append(0.5 * o[:T, :F_OUT] + 0.5)
    return np.stack(outs, 0)


def kernel(k, q, W_score, W_enh, b_enh, W_mask, b_mask):
    in_maps = make_in_maps(k, q, W_score, W_enh, b_enh, W_mask, b_mask)
    res = run_bass_kernel_spmd(get_nc(), in_maps, list(range(N_CORES)))
    return postprocess(res.results)
